# revision 5
# baseline (speedup 1.0000x reference)
"""Trainium2 Bass kernel for nn_GCNSet2SetNoGCNNet (Set2Set pooling + MLP).

Strategy (data-parallel over graphs, 8 NeuronCores):
  - 1024 graphs -> 128 graphs per core. Every graph padded to PMAX=1280 nodes
    (10 tiles of 128 nodes = 5 "pair tiles" of 256 nodes), so the whole
    program structure is static and identical across cores (SPMD).
  - Node features x are pre-packed on host into two bf16 layouts per core:
      xt: feature-major pair tiles [128(=2x64 feats), pair*128 + node]
      pn: node-major   pair tiles [128(node), pair*128 + 64*half + feat]
    xt stays SBUF-resident; pn is streamed from DRAM each step.
  - Per Set2Set step: LSTM cell (feature-major, tiny PE matmuls), then per
    pair tile: e = x.q via PE (stationary = xt pair, moving = packed q),
    softmax without max-subtraction (exp(e) is safe in f32 for this data;
    softmax is shift-invariant), denominator via two small PE matmuls
    (column sums + segment-select), exact correction for zero-padded nodes
    (each contributes exp(0)=1), r = sum a_i x_i via PE (stationary = pn
    pair, moving = attention weights), then q_star assembled with small
    permutation matmuls. Final 2-layer MLP on-chip.

Self-contained: hardcodes shapes/sharding; builds + compiles the Bass
program and runs it on cores 0-7 via run_bass_kernel_spmd.
"""

import numpy as np
import sys
for _p in ("/root/.axon_site/_ro/trn_rl_repo", "/opt/trn_rl_repo"):
    if _p not in sys.path:
        sys.path.append(_p)

import ml_dtypes

bf16 = ml_dtypes.bfloat16

# problem constants
B = 1024          # graphs
H = 64            # feature / hidden dim
STEPS = 5
PROJ = 128
NCORES = 8
G = B // NCORES   # 128 graphs per core
PMAX = 1280       # padded nodes per graph
TPG = PMAX // 128   # 10 tiles / graph
PPG = TPG // 2      # 5 pairs / graph
NPAIR = G * PPG     # 640 pairs / core
NCHUNK = 16
GCH = G // NCHUNK       # 8 graphs / chunk
PCH = NPAIR // NCHUNK   # 40 pairs / chunk
CCH = 2 * PCH           # 80 e/a columns / chunk
XTW = NPAIR * 128       # 81920 columns in xt/pn


# ---------------------------------------------------------------- host side

def host_prepack(x, batch):
    """Pad/scatter x into the per-core xt (feature-major) and pn (node-major)
    pair-tile layouts, plus per-graph negative pad counts."""
    x = np.ascontiguousarray(np.asarray(x), dtype=np.float32)
    batch = np.asarray(batch).astype(np.int64)
    counts = np.bincount(batch, minlength=B)
    assert counts.max() <= PMAX, f"graph with {counts.max()} nodes > PMAX={PMAX}"
    seg_start = np.zeros(B, np.int64)
    seg_start[1:] = np.cumsum(counts)[:-1]
    pos = np.arange(len(batch)) - seg_start[batch] + batch * PMAX
    xp = np.zeros((B * PMAX, H), np.float32)
    xp[pos] = x
    xp = xp.reshape(NCORES, G, PPG, 2, 128, H)     # [core,g,pair,half,node,feat]
    xt = np.transpose(xp, (0, 3, 5, 1, 2, 4))      # [core,half,feat,g,pair,node]
    xt = np.ascontiguousarray(xt.reshape(NCORES, 128, XTW)).astype(bf16)
    pn = np.transpose(xp, (0, 4, 1, 2, 3, 5))      # [core,node,g,pair,half,feat]
    pn = np.ascontiguousarray(pn.reshape(NCORES, 128, XTW)).astype(bf16)
    negnpad = -(PMAX - counts).reshape(NCORES, G, 1).astype(np.float32)
    return xt, pn, negnpad


def host_weights(W_ih, W_hh, b_ih, b_hh, W1, b1, W2, b2):
    """Reorder LSTM gates (i,f,g,o) -> (i,g,f,o) and transpose everything
    into the feature-major stationary layouts the kernel uses."""
    W_ih = np.asarray(W_ih, np.float32)
    W_hh = np.asarray(W_hh, np.float32)
    b = (np.asarray(b_ih, np.float32) + np.asarray(b_hh, np.float32))
    perm = np.concatenate([np.arange(0, 64), np.arange(128, 192),
                           np.arange(64, 128), np.arange(192, 256)])
    wih_t = np.ascontiguousarray(W_ih[perm].T)          # [128, 256]
    whh_t = np.ascontiguousarray(W_hh[perm].T)          # [64, 256]
    bias4 = np.ascontiguousarray(b[perm].reshape(4, 64).T)  # [64, 4]
    W1 = np.asarray(W1, np.float32)
    w1_t = np.ascontiguousarray(W1.T)                   # [128, 256]
    b1 = np.asarray(b1, np.float32)
    b1a = np.ascontiguousarray(b1[0:128, None])
    b1b = np.ascontiguousarray(b1[128:256, None])
    W2t = np.ascontiguousarray(np.asarray(W2, np.float32).T)  # [256, 128]
    w2ta = np.ascontiguousarray(W2t[0:128])
    w2tb = np.ascontiguousarray(W2t[128:256])
    b2c = np.ascontiguousarray(np.asarray(b2, np.float32)[:, None])
    return dict(wih_t=wih_t, whh_t=whh_t, bias4=bias4, w1_t=w1_t,
                b1a=b1a, b1b=b1b, w2ta=w2ta, w2tb=w2tb, b2=b2c)


def _consts():
    seg = np.zeros((CCH, NCHUNK * 128), np.float32)
    for c in range(NCHUNK):
        for cc in range(CCH):
            seg[cc, 128 * c + GCH * c + cc // TPG] = 1.0
    folda = np.zeros((128, 128), np.float32)   # [p, 64+p]=1 for p<64
    foldb = np.zeros((128, 128), np.float32)   # [64+p, 64+p]=1 for p<64
    stackh = np.zeros((64, 128), np.float32)   # [p, p]=1
    for p in range(64):
        folda[p, 64 + p] = 1.0
        foldb[64 + p, 64 + p] = 1.0
        stackh[p, p] = 1.0
    iden = np.eye(128, dtype=np.float32)
    ones_bf = np.ones((128, 1), bf16)
    onesrow = np.ones((1, 128), np.float32)
    return seg, folda, foldb, stackh, iden, ones_bf, onesrow


# ------------------------------------------------------------- bass program

def build_program():
    import concourse.bass as bass
    from concourse import bacc, mybir, tile

    f32 = mybir.dt.float32
    bft = mybir.dt.bfloat16
    AF = mybir.ActivationFunctionType
    ALU = mybir.AluOpType

    nc = bacc.Bacc("TRN2", target_bir_lowering=False, debug=False,
                   num_devices=NCORES)

    xt_d = nc.dram_tensor("xt", [128, XTW], bft, kind="ExternalInput")
    pn_d = nc.dram_tensor("pn", [128, XTW], bft, kind="ExternalInput")
    wih_d = nc.dram_tensor("wih_t", [128, 256], f32, kind="ExternalInput")
    whh_d = nc.dram_tensor("whh_t", [64, 256], f32, kind="ExternalInput")
    bias4_d = nc.dram_tensor("bias4", [64, 4], f32, kind="ExternalInput")
    w1_d = nc.dram_tensor("w1_t", [128, 256], f32, kind="ExternalInput")
    b1a_d = nc.dram_tensor("b1a", [128, 1], f32, kind="ExternalInput")
    b1b_d = nc.dram_tensor("b1b", [128, 1], f32, kind="ExternalInput")
    w2ta_d = nc.dram_tensor("w2ta", [128, 128], f32, kind="ExternalInput")
    w2tb_d = nc.dram_tensor("w2tb", [128, 128], f32, kind="ExternalInput")
    b2_d = nc.dram_tensor("b2", [128, 1], f32, kind="ExternalInput")
    negnpad_d = nc.dram_tensor("negnpad", [128, 1], f32, kind="ExternalInput")
    out_d = nc.dram_tensor("out", [128, 128], f32, kind="ExternalOutput")

    seg_np, folda_np, foldb_np, stackh_np, iden_np, ones_np, onesrow_np = _consts()
    seg_c = nc.inline_tensor(seg_np, "seg_c")
    folda_c = nc.inline_tensor(folda_np, "folda_c")
    foldb_c = nc.inline_tensor(foldb_np, "foldb_c")
    stackh_c = nc.inline_tensor(stackh_np, "stackh_c")
    iden_c = nc.inline_tensor(iden_np, "iden_c")
    ones_c = nc.inline_tensor(ones_np, "ones_c")
    onesrow_c = nc.inline_tensor(onesrow_np, "onesrow_c")

    with tile.TileContext(nc) as tc:
        with (
            tc.tile_pool(name="xtp", bufs=1) as xtp,
            tc.tile_pool(name="wp", bufs=1) as wp,
            tc.tile_pool(name="stp", bufs=2) as stp,
            tc.tile_pool(name="ap_", bufs=2) as ap_,
            tc.tile_pool(name="qp", bufs=2) as qp,
            tc.tile_pool(name="cp", bufs=2) as cp,
            tc.tile_pool(name="hp", bufs=2) as hp,
            tc.tile_pool(name="tp", bufs=1) as tp,
            tc.tile_pool(name="rrp", bufs=1) as rrp,
            tc.tile_pool(name="q2p", bufs=1) as q2p,
            tc.tile_pool(name="pe", bufs=2, space="PSUM") as pe_pool,
            tc.tile_pool(name="pr", bufs=1, space="PSUM") as pr_pool,
            tc.tile_pool(name="pd", bufs=1, space="PSUM") as pd_pool,
            tc.tile_pool(name="psm", bufs=2, space="PSUM") as psm_pool,
            tc.tile_pool(name="pl", bufs=2, space="PSUM") as pl_pool,
        ):
            # ---- resident loads
            xt_tiles = []
            for c in range(NCHUNK):
                xt_t = xtp.tile([128, PCH * 128], bft, name=f"xt_{c}")
                nc.sync.dma_start(xt_t[:, :], xt_d[:, c * PCH * 128:(c + 1) * PCH * 128])
                xt_tiles.append(xt_t)
            wih_sb = wp.tile([128, 256], f32, name="wih_sb")
            nc.sync.dma_start(wih_sb[:, :], wih_d[:, :])
            whh_sb = wp.tile([64, 256], f32, name="whh_sb")
            nc.sync.dma_start(whh_sb[:, :], whh_d[:, :])
            bias4_sb = wp.tile([64, 4], f32, name="bias4_sb")
            nc.sync.dma_start(bias4_sb[:, :], bias4_d[:, :])
            w1_sb = wp.tile([128, 256], f32, name="w1_sb")
            nc.sync.dma_start(w1_sb[:, :], w1_d[:, :])
            b1a_sb = wp.tile([128, 1], f32, name="b1a_sb")
            nc.sync.dma_start(b1a_sb[:, :], b1a_d[:, :])
            b1b_sb = wp.tile([128, 1], f32, name="b1b_sb")
            nc.sync.dma_start(b1b_sb[:, :], b1b_d[:, :])
            w2ta_sb = wp.tile([128, 128], f32, name="w2ta_sb")
            nc.sync.dma_start(w2ta_sb[:, :], w2ta_d[:, :])
            w2tb_sb = wp.tile([128, 128], f32, name="w2tb_sb")
            nc.sync.dma_start(w2tb_sb[:, :], w2tb_d[:, :])
            b2_sb = wp.tile([128, 1], f32, name="b2_sb")
            nc.sync.dma_start(b2_sb[:, :], b2_d[:, :])
            negnpad_sb = wp.tile([128, 1], f32, name="negnpad_sb")
            nc.sync.dma_start(negnpad_sb[:, :], negnpad_d[:, :])
            seg_sb = wp.tile([CCH, NCHUNK * 128], f32, name="seg_sb")
            nc.sync.dma_start(seg_sb[:, :], seg_c[:, :])
            folda_sb = wp.tile([128, 128], f32, name="folda_sb")
            nc.sync.dma_start(folda_sb[:, :], folda_c[:, :])
            foldb_sb = wp.tile([128, 128], f32, name="foldb_sb")
            nc.sync.dma_start(foldb_sb[:, :], foldb_c[:, :])
            stackh_sb = wp.tile([64, 128], f32, name="stackh_sb")
            nc.sync.dma_start(stackh_sb[:, :], stackh_c[:, :])
            iden_sb = wp.tile([128, 128], f32, name="iden_sb")
            nc.sync.dma_start(iden_sb[:, :], iden_c[:, :])
            ones_sb = wp.tile([128, 1], bft, name="ones_sb")
            nc.sync.dma_start(ones_sb[:, :], ones_c[:, :])
            onesrow_sb = wp.tile([1, 128], f32, name="onesrow_sb")
            nc.sync.dma_start(onesrow_sb[:, :], onesrow_c[:, :])

            # ---- state init
            q_star = qp.tile([128, G], f32, name="q_star")
            nc.vector.memset(q_star[:, :], 0.0)
            c_prev = cp.tile([64, G], f32, name="c_st")
            nc.vector.memset(c_prev[:, :], 0.0)
            Q2 = q2p.tile([128, 2 * G], bft, name="Q2")
            nc.vector.memset(Q2[:, :], 0.0)

            for s in range(STEPS):
                # ---------------- LSTM cell (gate order i, g, f, o)
                gates = []
                for k, fn in enumerate((AF.Sigmoid, AF.Tanh, AF.Sigmoid, AF.Sigmoid)):
                    pg = pl_pool.tile([64, G], f32, name="pg", tag="pl")
                    nc.tensor.matmul(pg[:, :], wih_sb[:, 64 * k:64 * k + 64],
                                     q_star[:, :], start=True, stop=False)
                    nc.tensor.matmul(pg[:, :], whh_sb[:, 64 * k:64 * k + 64],
                                     q_star[0:64, :], start=False, stop=True)
                    gsb = tp.tile([64, G], f32, name=f"gate{k}")
                    nc.scalar.activation(gsb[:, :], pg[:, :], fn,
                                         bias=bias4_sb[:, k:k + 1])
                    gates.append(gsb)
                g_i, g_g, g_f, g_o = gates
                fc = tp.tile([64, G], f32, name="fc")
                nc.vector.tensor_mul(fc[:, :], g_f[:, :], c_prev[:, :])
                ig = tp.tile([64, G], f32, name="ig")
                nc.vector.tensor_mul(ig[:, :], g_i[:, :], g_g[:, :])
                c_new = cp.tile([64, G], f32, name="c_st")
                nc.vector.tensor_add(c_new[:, :], fc[:, :], ig[:, :])
                tc_t = tp.tile([64, G], f32, name="tc_t")
                nc.scalar.activation(tc_t[:, :], c_new[:, :], AF.Tanh)
                h_sb = hp.tile([64, G], f32, name="h_sb")
                nc.vector.tensor_mul(h_sb[:, :], g_o[:, :], tc_t[:, :])
                c_prev = c_new

                # ---------------- pack q into Q2 (even cols top half,
                # odd cols bottom half; zeros elsewhere from initial memset)
                nc.vector.tensor_copy(Q2[0:64, 0:2 * G:2], h_sb[:, :])
                ph2 = pl_pool.tile([128, G], f32, name="ph2", tag="pl")
                nc.tensor.matmul(ph2[:, :], folda_sb[0:64, :], h_sb[:, :],
                                 start=True, stop=True)
                nc.vector.tensor_copy(Q2[64:128, 1:2 * G:2], ph2[64:128, :])

                # ---------------- attention passes, chunked
                psum_r = pr_pool.tile([128, 2 * G], f32, name="psum_r")
                psum_dall = pd_pool.tile([128, NCHUNK], f32, name="psum_dall")
                pending = None

                def emit_r_denom(pack):
                    c, stage_c, a_c = pack
                    for jj in range(PCH):
                        g_loc = GCH * c + jj // PPG
                        nc.tensor.matmul(
                            psum_r[:, 2 * g_loc:2 * g_loc + 2],
                            stage_c[:, jj * 128:(jj + 1) * 128],
                            a_c[:, 2 * jj:2 * jj + 2],
                            start=(jj % PPG == 0), stop=(jj % PPG == PPG - 1))
                    pcs = psm_pool.tile([CCH, 1], f32, name="pcs", tag="sm")
                    nc.tensor.matmul(pcs[:, :], a_c[:, :], ones_sb[:, :],
                                     start=True, stop=True)
                    cs_sb = tp.tile([CCH, 1], f32, name="cs_sb")
                    nc.vector.tensor_copy(cs_sb[:, :], pcs[:, :])
                    nc.tensor.matmul(psum_dall[:, c:c + 1],
                                     seg_sb[:, 128 * c:128 * (c + 1)],
                                     cs_sb[:, :], start=True, stop=True)

                for c in range(NCHUNK):
                    stage_c = stp.tile([128, PCH * 128], bft, name="stage")
                    nc.sync.dma_start(
                        stage_c[:, :], pn_d[:, c * PCH * 128:(c + 1) * PCH * 128])
                    pe_c = pe_pool.tile([128, CCH], f32, name="pe_c")
                    for jj in range(PCH):
                        g_loc = GCH * c + jj // PPG
                        nc.tensor.matmul(
                            pe_c[:, 2 * jj:2 * jj + 2],
                            xt_tiles[c][:, jj * 128:(jj + 1) * 128],
                            Q2[:, 2 * g_loc:2 * g_loc + 2],
                            start=True, stop=True)
                    a_c = ap_.tile([128, CCH], bft, name="a_c")
                    nc.scalar.activation(a_c[:, :], pe_c[:, :], AF.Exp)
                    if pending is not None:
                        emit_r_denom(pending)
                    pending = (c, stage_c, a_c)
                emit_r_denom(pending)

                # ---------------- denominators -> reciprocal (row layout)
                dsum = tp.tile([128, 1], f32, name="dsum")
                nc.vector.tensor_reduce(dsum[:, :], psum_dall[:, :],
                                        mybir.AxisListType.X, ALU.add)
                dcor = tp.tile([128, 1], f32, name="dcor")
                nc.vector.tensor_add(dcor[:, :], dsum[:, :], negnpad_sb[:, :])
                dclamp = tp.tile([128, 1], f32, name="dclamp")
                nc.vector.tensor_scalar_max(dclamp[:, :], dcor[:, :], 1e-30)
                rden = tp.tile([128, 1], f32, name="rden")
                nc.vector.reciprocal(rden[:, :], dclamp[:, :])
                prdt = psm_pool.tile([1, G], f32, name="prdt", tag="sm")
                nc.tensor.matmul(prdt[:, :], rden[:, :], iden_sb[:, :],
                                 start=True, stop=True)
                rdt_sb = tp.tile([1, G], f32, name="rdt_sb")
                nc.vector.tensor_copy(rdt_sb[:, :], prdt[:, :])

                # ---------------- r: evacuate, normalize, assemble q_star
                rr = rrp.tile([128, 2 * G], f32, name="rr")
                nc.vector.tensor_copy(rr[:, :], psum_r[:, :])
                pbc = psm_pool.tile([128, G], f32, name="pbc", tag="sm")
                nc.tensor.matmul(pbc[:, :], onesrow_sb[:, :], rdt_sb[:, :],
                                 start=True, stop=True)
                rr_s = rrp.tile([128, 2 * G], f32, name="rr_s")
                rdt_b = pbc[:, :].rearrange("p (g o) -> p g o", o=1)                    .broadcast_to((128, G, 2))
                nc.vector.tensor_tensor(
                    rr_s[:, :].rearrange("p (g t) -> p g t", t=2),
                    rr[:, :].rearrange("p (g t) -> p g t", t=2),
                    rdt_b, ALU.mult)
                pq = pl_pool.tile([128, G], f32, name="pq", tag="pl")
                nc.tensor.matmul(pq[:, :], stackh_sb[:, :], h_sb[:, :],
                                 start=True, stop=False)
                nc.tensor.matmul(pq[:, :], folda_sb[:, :], rr_s[:, 0:2 * G:2],
                                 start=False, stop=False)
                nc.tensor.matmul(pq[:, :], foldb_sb[:, :], rr_s[:, 1:2 * G:2],
                                 start=False, stop=True)
                q_star = qp.tile([128, G], f32, name="q_star")
                nc.vector.tensor_copy(q_star[:, :], pq[:, :])

            # ---------------- MLP head
            pz1a = pl_pool.tile([128, G], f32, name="pz1a", tag="pl")
            nc.tensor.matmul(pz1a[:, :], w1_sb[:, 0:128], q_star[:, :],
                             start=True, stop=True)
            z1a = tp.tile([128, G], f32, name="z1a")
            nc.scalar.activation(z1a[:, :], pz1a[:, :], AF.Relu,
                                 bias=b1a_sb[:, 0:1])
            pz1b = pl_pool.tile([128, G], f32, name="pz1b", tag="pl")
            nc.tensor.matmul(pz1b[:, :], w1_sb[:, 128:256], q_star[:, :],
                             start=True, stop=True)
            z1b = tp.tile([128, G], f32, name="z1b")
            nc.scalar.activation(z1b[:, :], pz1b[:, :], AF.Relu,
                                 bias=b1b_sb[:, 0:1])
            pout = pl_pool.tile([128, G], f32, name="pout", tag="pl")
            nc.tensor.matmul(pout[:, :], w2ta_sb[:, :], z1a[:, :],
                             start=True, stop=False)
            nc.tensor.matmul(pout[:, :], w2tb_sb[:, :], z1b[:, :],
                             start=False, stop=True)
            out_sb = tp.tile([128, G], f32, name="out_sb")
            nc.vector.tensor_scalar_add(out_sb[:, :], pout[:, :], b2_sb[:, 0:1])
            nc.sync.dma_start(out_d[:, :], out_sb[:, :])

    nc.compile()
    return nc


# ----------------------------------------------------------------- running

_NC_CACHE = None


def _get_program():
    global _NC_CACHE
    if _NC_CACHE is None:
        _NC_CACHE = build_program()
    return _NC_CACHE


def run(inputs, trace=False):
    from concourse.bass_utils import run_bass_kernel_spmd

    xt, pn, negnpad = host_prepack(inputs["x"], inputs["batch"])
    w = host_weights(inputs["W_ih"], inputs["W_hh"], inputs["b_ih"],
                     inputs["b_hh"], inputs["W1"], inputs["b1"],
                     inputs["W2"], inputs["b2"])
    in_maps = []
    for cc in range(NCORES):
        m = {"xt": xt[cc], "pn": pn[cc], "negnpad": negnpad[cc]}
        m.update(w)
        in_maps.append(m)
    nc = _get_program()
    res = run_bass_kernel_spmd(nc, in_maps, list(range(NCORES)), trace=trace)
    out = np.empty((B, PROJ), np.float32)
    for cc in range(NCORES):
        out[cc * G:(cc + 1) * G, :] = np.asarray(res.results[cc]["out"]).T
    return out, res


def kernel(**inputs) -> np.ndarray:
    out, _ = run(inputs, trace=False)
    return out
